# revision 6
# baseline (speedup 1.0000x reference)
"""TimeSformer-style attention on 8 TRN2 NeuronCores.

Sharding: core i -> batch b = i//2, head-group g = i%2 (6 of 12 heads).
Each core computes qkv projection for its 6 heads, full attention with
relative position+temporal bias, and a partial output projection.
Host sums the two partials per batch and adds proj_b.
"""

import numpy as np
import ml_dtypes

import concourse.bass as bass
import concourse.mybir as mybir
import concourse.tile as tile
from concourse import bacc
from concourse.bass_utils import run_bass_kernel_spmd
from concourse.masks import make_identity

WH, WW = 14, 14
NF = 8
DIM = 768
HEADS = 12
B = 4
N = (WH * WW + 1) * NF  # 1576
HD = DIM // HEADS  # 64
HG = HEADS // 2  # 6 heads per group
P = 128
NT = (N + P - 1) // P  # 13 row tiles (12x128 + 40)
KT = DIM // P  # 6 contraction tiles for qkv
CHUNKS = [(0, 512), (512, 512), (1024, 512), (1536, N - 1536)]
BF16 = mybir.dt.bfloat16
F32 = mybir.dt.float32
VW = HG * (HD + 1)  # 390: v with a ones-column per head

_CACHED = {}


def _rel_pos_index(wh, ww):
    nrd = (2 * wh - 1) * (2 * ww - 1) + 3
    area = wh * ww
    coords = np.stack(np.meshgrid(np.arange(wh), np.arange(ww), indexing='ij'))
    cf = coords.reshape(2, -1)
    rel = cf[:, :, None] - cf[:, None, :]
    rel = rel.transpose(1, 2, 0).copy()
    rel[:, :, 0] += wh - 1
    rel[:, :, 1] += ww - 1
    rel[:, :, 0] *= 2 * ww - 1
    idx = np.zeros((area + 1, area + 1), dtype=np.int64)
    idx[1:, 1:] = rel.sum(-1)
    idx[0, :] = nrd - 3
    idx[:, 0] = nrd - 2
    idx[0, 0] = nrd - 1
    return idx


def _rel_temp_index(nf):
    c = np.arange(nf)
    return (c[:, None] - c[None, :]) + nf - 1


def build_bass():
    nc = bacc.Bacc(None, target_bir_lowering=False)

    xt_d = nc.declare_dram_parameter("xt", [DIM, N], BF16, isOutput=False)
    wq_d = nc.declare_dram_parameter("wq", [DIM, HG * HD], BF16, isOutput=False)
    wk_d = nc.declare_dram_parameter("wk", [DIM, HG * HD], BF16, isOutput=False)
    wv_d = nc.declare_dram_parameter("wv", [DIM, VW], BF16, isOutput=False)
    bq_d = nc.declare_dram_parameter("bq", [1, HG * HD], BF16, isOutput=False)
    bv_d = nc.declare_dram_parameter("bv", [1, VW], BF16, isOutput=False)
    bias_d = nc.declare_dram_parameter("bias", [HG, N, N], BF16, isOutput=False)
    wp_d = nc.declare_dram_parameter("wp", [HG * HD, DIM], BF16, isOutput=False)
    out_d = nc.declare_dram_parameter("out", [N, DIM], F32, isOutput=True)

    with tile.TileContext(nc) as tc:
        with (
            tc.tile_pool(name="const", bufs=1) as const,
            tc.tile_pool(name="work", bufs=3) as work,
            tc.tile_pool(name="spool", bufs=1) as spool,
            tc.tile_pool(name="psA", bufs=1, space="PSUM") as psA,
            tc.tile_pool(name="psB", bufs=2, space="PSUM") as psB,
            tc.tile_pool(name="psT", bufs=2, space="PSUM") as psT,
        ):
            ident = const.tile([P, P], BF16, tag="ident")
            make_identity(nc, ident[:])
            ones = const.tile([1, 512], BF16, tag="ones")
            nc.gpsimd.memset(ones[:], 1.0)

            xt_sb = []
            wq_sb = []
            wk_sb = []
            wv_sb = []
            for k in range(KT):
                t = const.tile([P, N], BF16, tag=f"xt{k}")
                nc.sync.dma_start(t[:], xt_d[k * P:(k + 1) * P, :])
                xt_sb.append(t)
                t = const.tile([P, HG * HD], BF16, tag=f"wq{k}")
                nc.sync.dma_start(t[:], wq_d[k * P:(k + 1) * P, :])
                wq_sb.append(t)
                t = const.tile([P, HG * HD], BF16, tag=f"wk{k}")
                nc.sync.dma_start(t[:], wk_d[k * P:(k + 1) * P, :])
                wk_sb.append(t)
                t = const.tile([P, VW], BF16, tag=f"wv{k}")
                nc.sync.dma_start(t[:], wv_d[k * P:(k + 1) * P, :])
                wv_sb.append(t)
            bq_sb = const.tile([1, HG * HD], BF16, tag="bq")
            nc.sync.dma_start(bq_sb[:], bq_d[:])
            bv_sb = const.tile([1, VW], BF16, tag="bv")
            nc.sync.dma_start(bv_sb[:], bv_d[:])
            wp_sb = []
            for m in range(3):
                t = const.tile([P, DIM], BF16, tag=f"wp{m}")
                nc.sync.dma_start(t[:], wp_d[m * P:(m + 1) * P, :])
                wp_sb.append(t)

            # ---- qT, kT : [384, N] as 3 partition tiles, channel-major ----
            qT_sb = [const.tile([P, N], BF16, tag=f"qT{m}", name=f"qT{m}") for m in range(3)]
            kT_sb = [const.tile([P, N], BF16, tag=f"kT{m}", name=f"kT{m}") for m in range(3)]
            for m in range(3):
                for (c0, cw) in CHUNKS:
                    ps = psB.tile([P, 512], F32, tag="psB")
                    for k in range(KT):
                        nc.tensor.matmul(
                            ps[:, :cw],
                            wq_sb[k][:, m * P:(m + 1) * P],
                            xt_sb[k][:, c0:c0 + cw],
                            start=(k == 0), stop=False,
                        )
                    nc.tensor.matmul(
                        ps[:, :cw],
                        bq_sb[0:1, m * P:(m + 1) * P],
                        ones[0:1, :cw],
                        start=False, stop=True,
                    )
                    nc.scalar.copy(qT_sb[m][:, c0:c0 + cw], ps[:, :cw])
                for (c0, cw) in CHUNKS:
                    ps = psB.tile([P, 512], F32, tag="psB")
                    for k in range(KT):
                        nc.tensor.matmul(
                            ps[:, :cw],
                            wk_sb[k][:, m * P:(m + 1) * P],
                            xt_sb[k][:, c0:c0 + cw],
                            start=(k == 0), stop=(k == KT - 1),
                        )
                    nc.scalar.copy(kT_sb[m][:, c0:c0 + cw], ps[:, :cw])

            # ---- v natural [N, 390] as 13 row tiles (ones col per head) ----
            v_sb = []
            for nt in range(NT):
                pn = min(P, N - nt * P)
                t = const.tile([P, VW], BF16, tag=f"v{nt}")
                ps = psB.tile([P, 512], F32, tag="psB")
                for k in range(KT):
                    nc.tensor.matmul(
                        ps[:pn, :VW],
                        xt_sb[k][:, nt * P:nt * P + pn],
                        wv_sb[k][:],
                        start=(k == 0), stop=False,
                    )
                nc.tensor.matmul(
                    ps[:pn, :VW],
                    ones[0:1, :pn],
                    bv_sb[0:1, :],
                    start=False, stop=True,
                )
                nc.scalar.copy(t[:pn, :], ps[:pn, :VW])
                v_sb.append(t)

            # ---- attention per head ----
            attnT_sb = [const.tile([P, N], BF16, tag=f"aT{m}", name=f"aT{m}") for m in range(3)]
            for j in range(HG):
                mt, po = j // 2, (j % 2) * HD
                s_tiles = []
                for kt in range(NT):
                    kn = min(P, N - kt * P)
                    ps = psA.tile([P, N], F32, tag="psS")
                    for (c0, cw) in CHUNKS:
                        nc.tensor.matmul(
                            ps[:kn, c0:c0 + cw],
                            kT_sb[mt][po:po + HD, kt * P:kt * P + kn],
                            qT_sb[mt][po:po + HD, c0:c0 + cw],
                            start=True, stop=True,
                        )
                    bt = work.tile([P, N], BF16, tag="bias")
                    nc.sync.dma_start(bt[:kn, :], bias_d[j, kt * P:kt * P + kn, :])
                    sb = spool.tile([P, N], BF16, tag=f"S{kt}")
                    nc.vector.tensor_tensor(
                        out=sb[:kn, :], in0=ps[:kn, :], in1=bt[:kn, :],
                        op=mybir.AluOpType.add,
                    )
                    nc.scalar.activation(
                        sb[:kn, :], sb[:kn, :],
                        mybir.ActivationFunctionType.Exp,
                    )
                    s_tiles.append((sb, kn))
                for qt in range(NT):
                    qn = min(P, N - qt * P)
                    pv = psB.tile([P, 512], F32, tag="psB")
                    for kt in range(NT):
                        sb, kn = s_tiles[kt]
                        nc.tensor.matmul(
                            pv[:qn, :HD + 1],
                            sb[:kn, qt * P:qt * P + qn],
                            v_sb[kt][:kn, j * (HD + 1):(j + 1) * (HD + 1)],
                            start=(kt == 0), stop=(kt == NT - 1),
                        )
                    rec = work.tile([P, 1], F32, tag="rec")
                    nc.vector.reciprocal(rec[:qn], pv[:qn, HD:HD + 1])
                    ao = work.tile([P, HD], BF16, tag="ao")
                    nc.vector.tensor_scalar_mul(ao[:qn, :], pv[:qn, :HD], rec[:qn])
                    pt = psT.tile([HD, P], BF16, tag="psT")
                    nc.tensor.transpose(pt[:HD, :qn], ao[:qn, :HD], ident[:qn, :qn])
                    nc.scalar.copy(
                        attnT_sb[mt][po:po + HD, qt * P:qt * P + qn],
                        pt[:HD, :qn],
                    )

            # ---- partial output projection [N, 768] ----
            for nt in range(NT):
                pn = min(P, N - nt * P)
                for (f0, fw) in [(0, 512), (512, 256)]:
                    ps = psB.tile([P, 512], F32, tag="psB")
                    for m in range(3):
                        nc.tensor.matmul(
                            ps[:pn, :fw],
                            attnT_sb[m][:, nt * P:nt * P + pn],
                            wp_sb[m][:, f0:f0 + fw],
                            start=(m == 0), stop=(m == 2),
                        )
                    ob = work.tile([P, 512], F32, tag="ob")
                    nc.scalar.copy(ob[:pn, :fw], ps[:pn, :fw])
                    nc.sync.dma_start(out_d[nt * P:nt * P + pn, f0:f0 + fw],
                                      ob[:pn, :fw])
    nc.compile()
    return nc


def _prep(qkv_w, q_bias, v_bias, rel_pos_table, rel_temp_table, proj_w, num_frames):
    """Per-head-group host-side weight prep; cached across calls."""
    scale = HD ** -0.5
    rpi = _rel_pos_index(WH, WW)
    rti = _rel_temp_index(num_frames)
    bf = ml_dtypes.bfloat16
    groups = []
    for g in range(2):
        h0 = g * HG
        sl = slice(h0 * HD, (h0 + HG) * HD)
        wq = (qkv_w[sl, :].T * scale).astype(bf)
        wk = qkv_w[DIM:][sl, :].T.astype(bf)
        wv_base = qkv_w[2 * DIM:][sl, :].T  # [768, 384]
        wv = np.zeros((DIM, VW), dtype=bf)
        bv = np.zeros((1, VW), dtype=bf)
        for jj in range(HG):
            wv[:, jj * (HD + 1):jj * (HD + 1) + HD] = wv_base[:, jj * HD:(jj + 1) * HD].astype(bf)
            bv[0, jj * (HD + 1):jj * (HD + 1) + HD] = v_bias[sl][jj * HD:(jj + 1) * HD].astype(bf)
            bv[0, jj * (HD + 1) + HD] = 1.0
        bq = (q_bias[sl] * scale).astype(bf).reshape(1, -1)
        bias = np.empty((HG, N, N), dtype=bf)
        rep = N // num_frames
        for jj in range(HG):
            h = h0 + jj
            Ph = rel_pos_table[rpi.reshape(-1), h].reshape(rpi.shape)
            Th = rel_temp_table[rti.reshape(-1), h].reshape(rti.shape)
            bn = np.tile(Ph, (num_frames, num_frames)) \
                + np.repeat(np.repeat(Th, rep, 0), rep, 1)
            bias[jj] = bn.T.astype(bf)
        wp = proj_w[:, sl].T.astype(bf)  # [384, 768]
        groups.append(dict(wq=wq, wk=wk, wv=wv, bq=bq, bv=bv, bias=bias, wp=wp))
    return groups


def kernel(x, qkv_w, q_bias, v_bias, rel_pos_table, rel_temp_table, proj_w,
           proj_b, num_frames):
    x = np.asarray(x, dtype=np.float32)
    nf = int(num_frames)
    bf = ml_dtypes.bfloat16

    key = "nc"
    if key not in _CACHED:
        _CACHED[key] = build_bass()
    nc = _CACHED[key]

    groups = _prep(np.asarray(qkv_w, np.float32), np.asarray(q_bias, np.float32),
                   np.asarray(v_bias, np.float32),
                   np.asarray(rel_pos_table, np.float32),
                   np.asarray(rel_temp_table, np.float32),
                   np.asarray(proj_w, np.float32), nf)

    in_maps = []
    for i in range(8):
        b, g = i // 2, i % 2
        m = dict(groups[g])
        m["xt"] = np.ascontiguousarray(x[b].T).astype(bf)
        in_maps.append(m)

    res = run_bass_kernel_spmd(nc, in_maps, list(range(8)))
    parts = [r["out"] for r in res.results]
    out = np.empty((B, N, DIM), dtype=np.float32)
    pb = np.asarray(proj_b, np.float32)
    for b in range(B):
        out[b] = parts[2 * b] + parts[2 * b + 1] + pb
    return out
